# revision 17
# baseline (speedup 1.0000x reference)
"""Trainium2 Bass kernel for nn_MultiHeadCrossAttention (B,C,H,W = 8,512,64,64).

Self-contained: builds one single-core Bass/Tile program and runs it SPMD on
8 NeuronCores (data-parallel, one batch element per core).

v2: PE-quadrant-packed AV, bf16 V-projection (gpsimd cast pipeline),
DVE q-path (no Wq transposes), per-j2 softmax pipeline, staged output DMA.
"""
import sys

for _p in ("/opt/trn_rl_repo", "/root/.axon_site/_ro/trn_rl_repo"):
    if _p not in sys.path:
        sys.path.append(_p)

import numpy as np


# ---------------------------------------------------------------------------
# Workaround: this walrus build caps sync-waits per CTRL instruction; the
# TileContext exit drain accumulates one wait per active processor and blows
# the cap.  Pre-absorb each wait on its own SP nop before the drain.
# ---------------------------------------------------------------------------
def _install_drain_patch():
    import concourse.tile as tile
    from concourse.vector_clock import ScopedClock

    if getattr(tile.TileContext, "_drain_patch_installed", False):
        return

    def _patched(self, tick_clock, wait_clock):
        nc = self.nc
        gc = tick_clock.global_clock
        scoped = gc if hasattr(gc, "items") else ScopedClock({None: gc})
        for scope, clock in scoped.items():
            for i in range(32):
                try:
                    t = clock.peek_next(i) - 1
                except Exception:
                    break
                if t > 0:
                    nop = nc.sync.nop(nofuse=True, hint="drain_split")
                    sc = ScopedClock()
                    sc.require_at_least(scope, i, t)
                    wait_clock.add_sem_waits(nop.ins, sc)
        nc.sync.drain()  # nops above absorbed every wait; SP is in-order

        nc.all_engine_barrier()
        assert self.sems is not None
        popped = nc._tile_sem_poison_stack.pop()
        assert popped is self._sem_poison
        nc.clear_and_free_semaphores(list(self.sems.allocated().values()))
        nc.all_engine_barrier()

    tile.TileContext._drain_and_barrier = _patched
    tile.TileContext._drain_patch_installed = True


import concourse.bass as bass
import concourse.tile as tile
from concourse import mybir

F32 = mybir.dt.float32
F32R = mybir.dt.float32r
BF16 = mybir.dt.bfloat16
AF = mybir.ActivationFunctionType
ALU = mybir.AluOpType
AX = mybir.AxisListType

C, HW, NH, D, H, W = 512, 4096, 8, 64, 64, 64
NB = 4  # 128-partition blocks of C


def _split_excess_waits(nc, cap=2):
    """This walrus build caps sync-waits per ISA instruction.  Move excess
    waits onto same-engine NoOps inserted just before the instruction
    (same engine => executes immediately before it; semantically identical)."""
    k = 0
    for fn in nc.m.functions:
        for blk in fn.blocks:
            out, changed = [], False
            for inst in blk.instructions:
                si = inst.sync_info
                icap = 1
                if si is not None and len(si.on_wait) > icap:
                    waits = list(si.on_wait)
                    excess, keep = waits[:-icap], waits[-icap:]
                    while excess:
                        chunk, excess = excess[:1], excess[1:]
                        k += 1
                        nop = mybir.InstNoOp(
                            name=f"I-waitsplit-{k}", engine=inst.engine
                        )
                        nop.sync_info = mybir.SyncInfo(
                            on_wait=chunk, on_update=[]
                        )
                        nc.register_instruction(nop)
                        out.append(nop)
                    inst.sync_info = mybir.SyncInfo(
                        on_wait=keep, on_update=list(si.on_update)
                    )
                    changed = True
                out.append(inst)
            if changed:
                blk.instructions = out
    return k


def build():
    nc = bass.Bass("TRN2", target_bir_lowering=False, debug=False, num_devices=1)

    hidden = nc.dram_tensor("hidden", [C, HW], F32R, kind="ExternalInput")
    guide = nc.dram_tensor("guide", [1, C], F32, kind="ExternalInput")
    Wq = nc.dram_tensor("Wq", [C, C], F32, kind="ExternalInput")
    Wk = nc.dram_tensor("Wk", [C, C], F32, kind="ExternalInput")
    Wv = nc.dram_tensor("Wv", [C, C], F32, kind="ExternalInput")
    bq = nc.dram_tensor("bq", [1, C], F32, kind="ExternalInput")
    bk = nc.dram_tensor("bk", [1, C], F32, kind="ExternalInput")
    bv = nc.dram_tensor("bv", [1, C], F32, kind="ExternalInput")
    out = nc.dram_tensor("out", [C, HW], F32, kind="ExternalOutput")

    with tile.TileContext(nc) as tc:
        _body(nc, tc, hidden, guide, Wq, Wk, Wv, bq, bk, bv, out)
    _split_excess_waits(nc)
    return nc


def _body(nc, tc, hidden, guide, Wq, Wk, Wv, bq, bk, bv, out):
    import contextlib

    ctx = contextlib.ExitStack()
    with ctx:
        P = ctx.enter_context(tc.tile_pool(name="persist", bufs=1))
        WN = ctx.enter_context(tc.tile_pool(name="wnat", bufs=4))
        WQ = ctx.enter_context(tc.tile_pool(name="wqp", bufs=2))
        SC = ctx.enter_context(tc.tile_pool(name="scpool", bufs=2))
        ZS = ctx.enter_context(tc.tile_pool(name="zspool", bufs=2))
        ASB = ctx.enter_context(tc.tile_pool(name="attsb", bufs=2))
        PS = ctx.enter_context(tc.tile_pool(name="ps", bufs=2, space="PSUM"))
        PSM = ctx.enter_context(tc.tile_pool(name="psm", bufs=2, space="PSUM"))
        PZ = ctx.enter_context(tc.tile_pool(name="pz", bufs=2, space="PSUM"))

        # ---------------- constants ----------------
        ident = P.tile([128, 128], F32, tag="ident")
        from concourse.masks import make_identity

        make_identity(nc, ident[:])
        one1 = P.tile([1, 1], F32, tag="one1")
        nc.vector.memset(one1[:], 1.0)
        ones_r = P.tile([1, 128], F32, tag="ones_r")
        nc.vector.memset(ones_r[:], 1.0)
        # half-indicator rows for qv2 broadcast (K=1 matmuls)
        ones1a = P.tile([1, 128], F32, tag="ones1a")
        ones1b = P.tile([1, 128], F32, tag="ones1b")
        nc.vector.memset(ones1a[:], 0.0)
        nc.vector.memset(ones1a[:, 0:64], 1.0)
        nc.vector.memset(ones1b[:], 0.0)
        nc.vector.memset(ones1b[:, 64:128], 1.0)
        # column indicator [128,2] (lhsT for Z row sums over each 64-half)
        i2colsT = P.tile([128, 2], BF16, tag="i2colsT")
        nc.vector.memset(i2colsT[:], 0.0)
        nc.vector.memset(i2colsT[0:64, 0:1], 1.0)
        nc.vector.memset(i2colsT[64:128, 1:2], 1.0)
        # row indicator [2,128] (lhsT for partition-broadcast of zinv rows)
        i2colsF = P.tile([128, 2], F32, tag="i2colsF")
        nc.vector.memset(i2colsF[:], 0.0)
        nc.vector.memset(i2colsF[0:64, 0:1], 1.0)
        nc.vector.memset(i2colsF[64:128, 1:2], 1.0)
        i2rows = P.tile([2, 128], BF16, tag="i2rows")
        i2rp = PSM.tile([2, 128], F32, tag="small")
        nc.tensor.transpose(out=i2rp[:], in_=i2colsF[:], identity=ident[:])
        nc.vector.tensor_copy(i2rows[:], i2rp[:])

        # ---------------- small loads ----------------
        guide_sb = P.tile([1, C], F32, tag="guide_sb")
        nc.sync.dma_start(out=guide_sb, in_=guide.ap())
        brow = {}
        for nm, dr in (("bq", bq), ("bk", bk), ("bv", bv)):
            t = P.tile([1, C], F32, tag=f"{nm}_row")
            nc.sync.dma_start(out=t, in_=dr.ap())
            brow[nm] = t

        # weight naturals: Wv first (gates V-proj), Wk next, Wq persistent
        natv, natk = [], []
        for cb in range(NB):
            t = WN.tile([128, C], F32, tag="wnat")
            nc.sync.dma_start(out=t, in_=Wv.ap()[cb * 128:(cb + 1) * 128, :])
            natv.append(t)
        for cb in range(NB):
            t = WN.tile([128, C], F32, tag="wnat")
            nc.sync.dma_start(out=t, in_=Wk.ap()[cb * 128:(cb + 1) * 128, :])
            natk.append(t)
        # ---------------- bias columns (PE transpose of rows) ----------------
        bkcol, bvcol = [], []
        bqp_sb = P.tile([128, NB], F32, tag="bqcol")
        for j in range(NB):
            pt = PSM.tile([128, 3], F32, tag="small")
            nc.tensor.transpose(
                out=pt[:, 0:1], in_=brow["bk"][:, j * 128:(j + 1) * 128],
                identity=one1[:],
            )
            nc.tensor.transpose(
                out=pt[:, 1:2], in_=brow["bv"][:, j * 128:(j + 1) * 128],
                identity=one1[:],
            )
            nc.tensor.transpose(
                out=pt[:, 2:3], in_=brow["bq"][:, j * 128:(j + 1) * 128],
                identity=one1[:],
            )
            kc = P.tile([128, 1], F32, tag=f"bk64c{j}")
            vc = P.tile([128, 1], F32, tag=f"bvc{j}")
            nc.scalar.mul(kc[:], pt[:, 0:1], 64.0)  # K bias enters via 64-token sum
            nc.scalar.copy(vc[:], pt[:, 1:2])
            nc.vector.tensor_copy(bqp_sb[:, j:j + 1], pt[:, 2:3])
            bkcol.append(kc)
            bvcol.append(vc)

        # ---------------- weight transposes (PE), per source block ----------------
        # wT[i][p, c_out] = W[c_out, 128i+p]
        wvT = [P.tile([128, C], F32R, tag=f"wvT{i}", name=f"wvT{i}") for i in range(NB)]
        wkT = [P.tile([128, C], F32, tag=f"wkT{i}", name=f"wkT{i}") for i in range(NB)]
        for cb in range(NB):
            for i in range(NB):
                ptv = PSM.tile([128, 128], F32, tag="small")
                nc.tensor.transpose(
                    out=ptv[:], in_=natv[cb][:, i * 128:(i + 1) * 128],
                    identity=ident[:],
                )
                if (cb + i) % 2 == 0:
                    nc.vector.tensor_copy(wvT[i][:, cb * 128:(cb + 1) * 128], ptv[:])
                else:
                    nc.scalar.copy(wvT[i][:, cb * 128:(cb + 1) * 128], ptv[:])
        for cb in range(NB):
            for i in range(NB):
                ptk = PSM.tile([128, 128], F32, tag="small")
                nc.tensor.transpose(
                    out=ptk[:], in_=natk[cb][:, i * 128:(i + 1) * 128],
                    identity=ident[:],
                )
                if (cb + i) % 2 == 0:
                    nc.scalar.copy(wkT[i][:, cb * 128:(cb + 1) * 128], ptk[:])
                else:
                    nc.vector.tensor_copy(wkT[i][:, cb * 128:(cb + 1) * 128], ptk[:])

        # Wq naturals: loaded after the transposes consumed natv slots (WN ring)
        wq_nat = []
        for cb in range(NB):
            t = WQ.tile([128, C], F32, tag="wq", name=f"wqn{cb}")
            nc.sync.dma_start(out=t, in_=Wq.ap()[cb * 128:(cb + 1) * 128, :])
            wq_nat.append(t)

        # ------------- x load + G partial sums + V projection -------------
        x = [P.tile([128, HW], F32R, tag=f"x{i}", name=f"x{i}") for i in range(NB)]
        vt = [P.tile([128, HW], BF16, tag=f"vt{j}", name=f"vt{j}") for j in range(NB)]
        g = [P.tile([128, 64], F32, tag=f"g{i}", name=f"g{i}") for i in range(NB)]
        qv2 = []

        for t8 in range(8):
            for i in range(NB):
                nc.sync.dma_start(
                    out=x[i][:, t8 * 512:(t8 + 1) * 512],
                    in_=hidden.ap()[i * 128:(i + 1) * 128, t8 * 512:(t8 + 1) * 512],
                )
                # G: sum the 64 dd-tokens of this chunk per hu
                nc.vector.tensor_reduce(
                    out=g[i][:, t8 * 8:(t8 + 1) * 8],
                    in_=x[i][:, t8 * 512:(t8 + 1) * 512].bitcast(F32)
                        .rearrange("p (dd hu) -> p hu dd", hu=8),
                    axis=AX.X, op=ALU.add,
                )
            # V-proj for this token chunk
            for j in range(NB):
                pt = PS.tile([128, 512], F32, tag="b512")
                for i in range(NB):
                    nc.tensor.matmul(
                        out=pt[:],
                        lhsT=wvT[i][:, j * 128:(j + 1) * 128],
                        rhs=x[i][:, t8 * 512:(t8 + 1) * 512],
                        start=(i == 0), stop=(i == NB - 1),
                    )
                dst = vt[j][:, t8 * 512:(t8 + 1) * 512]
                nc.scalar.activation(dst, pt[:], AF.Identity, bias=bvcol[j][:])

            if t8 == 0:
                # ---- q path (DVE): q_row[c_out] = sum_c Wq[c_out,c]*guide[c] ----
                gbp = PS.tile([128, 512], F32, tag="b512")
                nc.tensor.matmul(
                    out=gbp[:], lhsT=ones_r[:], rhs=guide_sb[:],
                    start=True, stop=True,
                )
                qcol = P.tile([128, NB], F32, tag="qcol")
                for j in range(NB):
                    mq = SC.tile([128, 512], F32, tag="sc")
                    nc.vector.tensor_mul(mq[:], wq_nat[j][:], gbp[:])
                    nc.vector.tensor_reduce(
                        out=qcol[:, j:j + 1], in_=mq[:], axis=AX.X, op=ALU.add
                    )
                qcolb = P.tile([128, NB], F32, tag="qcolb")
                nc.vector.tensor_add(qcolb[:], qcol[:], bqp_sb[:])
                for j2 in range(NB):
                    qrp = PSM.tile([1, 128], F32, tag="small")
                    nc.tensor.transpose(
                        out=qrp[:], in_=qcolb[:, j2:j2 + 1], identity=ident[:]
                    )
                    qrow_sb = P.tile([1, 128], F32, tag=f"qrow{j2}")
                    nc.vector.tensor_copy(qrow_sb[:], qrp[:])
                    qp2 = PSM.tile([128, 64], F32, tag="small")
                    nc.tensor.matmul(
                        out=qp2[:], lhsT=ones1a[:], rhs=qrow_sb[:, 0:64],
                        start=True, stop=False,
                    )
                    nc.tensor.matmul(
                        out=qp2[:], lhsT=ones1b[:], rhs=qrow_sb[:, 64:128],
                        start=False, stop=True,
                    )
                    t = P.tile([128, 64], F32, tag=f"qv2_{j2}")
                    nc.scalar.mul(t[:], qp2[:], 0.125)  # fold scale = 1/sqrt(d)
                    qv2.append(t)

        # ---------------- SK = Wk @ G.T + 64*bk ----------------
        skT = []
        for j in range(NB):
            skp = PSM.tile([128, 64], F32, tag="small")
            for i in range(NB):
                nc.tensor.matmul(
                    out=skp[:], lhsT=wkT[i][:, j * 128:(j + 1) * 128],
                    rhs=g[i][:], start=(i == 0), stop=(i == NB - 1),
                )
            t = P.tile([128, 64], F32, tag=f"skT{j}")
            nc.vector.tensor_scalar_add(t[:], skp[:], bkcol[j][:])
            skT.append(t)

        # ---------------- scores (gpsimd) + exp (ACT) ----------------
        e = [P.tile([128, HW], BF16, tag=f"e{j2}", name=f"e{j2}") for j2 in range(NB)]
        for j2 in range(NB):
            for n in range(8):
                sct = SC.tile([128, 512], F32, tag="sc")
                in0 = (qv2[j2][:].rearrange("p (a q) -> p a q", a=1)
                       .broadcast_to((128, 8, 64)))
                in1 = skT[j2][:, n * 8:(n + 1) * 8].broadcast_to((128, 8, 64))
                o3 = sct[:].rearrange("p (s q) -> p s q", q=64)
                nc.vector.tensor_mul(o3, in0, in1)
                nc.scalar.activation(
                    e[j2][:, n * 512:(n + 1) * 512], sct[:], AF.Exp
                )

        # ---------------- softmax-normalize + AV, pipelined over j2 ----------------
        zr = P.tile([128, 256], F32, tag="zr")
        zinv = P.tile([128, 256], BF16, tag="zinv")

        def emit_zsum(j2):
            for cc in range(8):
                zp = PSM.tile([2, 512], F32, tag="small")
                nc.tensor.matmul(
                    out=zp[:], lhsT=i2colsT[:],
                    rhs=e[j2][:, cc * 512:(cc + 1) * 512],
                    start=True, stop=True,
                )
                zs = ZS.tile([2, 512], F32, tag="zs")
                if cc % 2 == 0:
                    nc.vector.tensor_copy(zs[:], zp[:])
                else:
                    nc.scalar.copy(zs[:], zp[:])
                for r2 in range(2):
                    rho = 2 * j2 + r2
                    nc.sync.dma_start(
                        out=zr[rho * 16 + 2 * cc:rho * 16 + 2 * cc + 2, :],
                        in_=zs[r2:r2 + 1, :].rearrange("p (c f) -> p c f", f=256),
                    )

        def emit_zinv_zir(j2):
            with nc.allow_low_precision(reason="bf16 zinv scales softmax"):
                nc.vector.reciprocal(
                    zinv[32 * j2:32 * j2 + 32, :], zr[32 * j2:32 * j2 + 32, :]
                )
            zir = ZS.tile([2, HW], BF16, tag="zir")
            for r2 in range(2):
                nc.sync.dma_start(
                    out=zir[r2:r2 + 1, :].rearrange("p (c f) -> p c f", f=256),
                    in_=zinv[32 * j2 + 16 * r2:32 * j2 + 16 * r2 + 16, :],
                )
            return zir

        def emit_av(j2, zir):
            for jp in range(NB):
                for n2x in range(2):
                    cc = 2 * jp + n2x
                    zb = PZ.tile([128, 512], F32, tag="zb")
                    nc.tensor.matmul(
                        out=zb[:], lhsT=i2rows[:],
                        rhs=zir[:, cc * 512:(cc + 1) * 512],
                        start=True, stop=True,
                    )
                    es = e[j2][:, cc * 512:(cc + 1) * 512]
                    nc.vector.tensor_mul(es, es, zb[:])
                at = [PS.tile([128, 512], F32, tag="b512", name=f"at{j2}_{jp}_{r2x}") for r2x in range(2)]
                for gg in range(8):
                    for n2 in range(2):
                        n = 2 * jp + n2
                        for r2 in range(2):
                            vmat = vt[j2][
                                r2 * 64:(r2 + 1) * 64,
                                n * 512 + gg:n * 512 + gg + 505:8,
                            ]
                            nc.tensor.matmul(
                                out=at[r2][n2 * 64:(n2 + 1) * 64,
                                           gg * 64:(gg + 1) * 64],
                                lhsT=vmat,
                                rhs=e[j2][
                                    r2 * 64:(r2 + 1) * 64,
                                    (8 * n + gg) * 64:(8 * n + gg + 1) * 64,
                                ],
                                start=True, stop=True,
                            )
                asb = ASB.tile([128, 1024], F32, tag="asb")
                for r2 in range(2):
                    dst = asb[:].rearrange(
                        "p (gg rr q) -> p rr gg q", rr=2, q=64
                    )[:, r2]
                    src = at[r2][:].rearrange("p (gg q) -> p gg q", q=64)
                    if r2 == 0:
                        nc.vector.tensor_copy(dst, src)
                    else:
                        nc.scalar.copy(dst, src)
                dmadst = out.ap()[jp * 128:(jp + 1) * 128, :].rearrange(
                    "p (gg j w) -> p j gg w", j=4, w=128
                )[:, j2]
                nc.gpsimd.dma_start(
                    out=dmadst, in_=asb[:].rearrange("p (gg w) -> p gg w", w=128)
                )

        # PE-order schedule: Zsum[j2+1] early; zb/norm interleaved inside AV
        emit_zsum(0)
        zir0 = emit_zinv_zir(0)
        emit_zsum(1)
        emit_av(0, zir0)
        zir1 = emit_zinv_zir(1)
        emit_zsum(2)
        emit_av(1, zir1)
        zir2 = emit_zinv_zir(2)
        emit_zsum(3)
        emit_av(2, zir2)
        zir3 = emit_zinv_zir(3)
        emit_av(3, zir3)


# ---------------------------------------------------------------------------
# Runner: full-input -> shard over 8 cores -> gather
# ---------------------------------------------------------------------------
_NC_CACHE = {}


def _get_nc():
    if "nc" not in _NC_CACHE:
        _install_drain_patch()
        _NC_CACHE["nc"] = build()
    return _NC_CACHE["nc"]


def run_sharded(inputs, trace=False, trace_kwargs=None):
    """inputs: full-size arrays keyed as in reference.setup_inputs()."""
    from concourse.bass_utils import run_bass_kernel_spmd

    guide = np.asarray(inputs["guide"], dtype=np.float32)
    hidden = np.asarray(inputs["hidden_rep"], dtype=np.float32)
    B = hidden.shape[0]
    assert B == 8 and hidden.shape[1:] == (C, H, W)
    Wq = np.asarray(inputs["Wq"], dtype=np.float32)
    Wk = np.asarray(inputs["Wk"], dtype=np.float32)
    Wv = np.asarray(inputs["Wv"], dtype=np.float32)
    bq = np.asarray(inputs["bq"], dtype=np.float32).reshape(1, C)
    bk = np.asarray(inputs["bk"], dtype=np.float32).reshape(1, C)
    bv = np.asarray(inputs["bv"], dtype=np.float32).reshape(1, C)

    in_maps = []
    for b in range(B):
        in_maps.append({
            "hidden": np.ascontiguousarray(hidden[b].reshape(C, HW)),
            "guide": np.ascontiguousarray(guide[b:b + 1]),
            "Wq": Wq, "Wk": Wk, "Wv": Wv,
            "bq": bq, "bk": bk, "bv": bv,
        })

    nc = _get_nc()
    kw = {}
    if trace:
        kw["trace"] = True
        kw.update(trace_kwargs or {})
    res = run_bass_kernel_spmd(nc, in_maps, list(range(B)), **kw)
    out = np.stack([res.results[b]["out"].reshape(C, H, W) for b in range(B)])
    return out.astype(np.float32), res


def kernel(**inputs):
    out, _ = run_sharded(inputs)
    return out
